# revision 1
# baseline (speedup 1.0000x reference)
"""Trainium2 Bass kernel for a bare KAN layer (PCHIP spline mixing).

Math: out[b, o] = sum_d f_{o,d}(x[b,d]) + bias[o], where f_{o,d} is the PCHIP
cubic interpolant of coeffs[o,d,:] on K=64 uniform knots over [-2, 2], with
linear extrapolation outside.

Device strategy (per core, data-parallel over batch):
  Any C^1 piecewise cubic on the knot grid is exactly
      f(t) = sum_k C[k]*alpha_k(t) + (h*S[k])*beta_k(t)
  in the Hermite cardinal basis
      alpha_k = w^2*(3-2w),  beta_k = (t-k)*w^2,  w = relu(1 - |t-k|)
  with t = (x-X_MIN)/h clamped to [0, K-1]; linear extrapolation outside the
  domain equals an extra  min(t,0)*hS[0] + max(t-(K-1),0)*hS[K-1]  term.

  Per input-dim d the kernel builds three [2K=128, B] fp16 "weight fields"
  w^2, w^3, w^2*(1+y) (w from two ACT relu passes off a PE-broadcast t) and
  contracts them in PSUM against fp16 tables 3C-hS / -2C / hS:
      w2*(3C-hS) + w3*(-2C) + w2*(1+y)*hS = C*alpha + hS*beta.
  The t-broadcast is an fp16 hi/lo split matmul (exact to ~1e-5).

Self-contained: hardcodes shapes B=8192, D=64, K=64, O=64, 8 cores.
"""

import sys

import numpy as np

sys.path.insert(0, "/opt/trn_rl_repo")

from concourse import bass, mybir  # noqa: E402
from concourse.bass_utils import run_bass_kernel_spmd  # noqa: E402
from concourse.tile import TileContext  # noqa: E402

F32 = mybir.dt.float32
F16 = mybir.dt.float16
ALU = mybir.AluOpType
AF = mybir.ActivationFunctionType

B, D, K, O = 8192, 64, 64, 64
NCORES = 8
BSH = B // NCORES          # 1024 batch rows per core
NCHUNK = 2                 # 512-column matmul chunks
CHUNK = BSH // NCHUNK      # 512
NPAIR = D // 2             # 32 d-pairs; each pair-tile has 128 = 2*64 rows
X_MIN, X_MAX = -2.0, 2.0
H = (X_MAX - X_MIN) / (K - 1)

# fp16 table tensor [128, CF16], interleaved per group:
# cols g*3*O + [0:O)=3C-hS, [O:2O)=-2C, [2O:3O)=hS
C16_T1 = 0
C16_T2 = O
C16_T3 = 2 * O
CF16 = NPAIR * 3 * O                    # 6144
GSPLIT = 8                              # groups in the first table DMA
# fp16 selector tensor [128, CS16]: bsel groups then etab
CS_BSEL = 0
CS_ETAB = CS_BSEL + NPAIR * 128         # 4096
CS16 = CS_ETAB + O                      # 4160

# fp32 const tensor [128, CF32]
C32_PK1 = 0                             # 1 + k
C32_MK1 = 1                             # 1 - k
C32_BIAS = 2
CF32 = 3

TRACE = False
LAST_EXEC_NS = None


def _pchip_slopes_uniform(y, h):
    """numpy float32 port of reference._pchip_slopes_uniform. y: [..., K]."""
    y = y.astype(np.float32)
    delta = ((y[..., 1:] - y[..., :-1]) / np.float32(h)).astype(np.float32)
    dp, dn = delta[..., :-1], delta[..., 1:]
    same_sign = dp * dn > 0
    d_mid = np.where(
        same_sign, (2.0 * dp * dn / (dp + dn + np.float32(1e-12))), np.float32(0.0)
    ).astype(np.float32)

    def _fix_endpoint(d_end, delta0, delta1):
        d_end = np.where(d_end * delta0 <= 0, np.float32(0.0), d_end)
        d_end = np.where(
            (delta0 * delta1 < 0) & (np.abs(d_end) > 3.0 * np.abs(delta0)),
            (3.0 * delta0).astype(np.float32),
            d_end,
        )
        return d_end.astype(np.float32)

    d0 = _fix_endpoint(
        ((3.0 * delta[..., 0] - delta[..., 1]) / 2.0).astype(np.float32),
        delta[..., 0],
        delta[..., 1],
    )
    dN = _fix_endpoint(
        ((3.0 * delta[..., -1] - delta[..., -2]) / 2.0).astype(np.float32),
        delta[..., -1],
        delta[..., -2],
    )
    return np.concatenate([d0[..., None], d_mid, dN[..., None]], axis=-1)


def _build_kernel(repeat=1):
    nc = bass.Bass()

    xt = nc.declare_dram_parameter("xt", [D, BSH], F32, isOutput=False)
    c16 = nc.declare_dram_parameter("c16", [128, CF16], F16, isOutput=False)
    cs16 = nc.declare_dram_parameter("cs16", [128, CS16], F16, isOutput=False)
    c32 = nc.declare_dram_parameter("c32", [128, CF32], F32, isOutput=False)
    outt = nc.declare_dram_parameter("outt", [O, BSH], F32, isOutput=True)

    with TileContext(nc) as tc:
        with (
            tc.tile_pool(name="consts", bufs=1) as consts,
            tc.tile_pool(name="work", bufs=6) as work,
            tc.tile_pool(name="tcbp", bufs=3, space="PSUM") as tcbp,
            tc.tile_pool(name="accp", bufs=1, space="PSUM") as accp,
        ):
            xt_sb = consts.tile([D, BSH], F32)
            c16_sb = consts.tile([128, CF16], F16)
            cs16_sb = consts.tile([128, CS16], F16)
            c32_sb = consts.tile([128, CF32], F32)
            nc.sync.dma_start(xt_sb[:], xt[:])
            nc.sync.dma_start(cs16_sb[:], cs16[:])
            nc.sync.dma_start(
                c16_sb[:, : GSPLIT * 3 * O], c16[:, : GSPLIT * 3 * O]
            )
            nc.sync.dma_start(
                c16_sb[:, GSPLIT * 3 * O :], c16[:, GSPLIT * 3 * O :]
            )
            nc.sync.dma_start(c32_sb[:], c32[:])

            def tab(base, g):
                lo = g * 3 * O + base
                return c16_sb[:, lo : lo + O]

            etab_t = cs16_sb[:, CS_ETAB : CS_ETAB + O]

            def bsel_t(g):
                return cs16_sb[:, CS_BSEL + g * 128 : CS_BSEL + (g + 1) * 128]

            pk1_t = c32_sb[:, C32_PK1 : C32_PK1 + 1]
            mk1_t = c32_sb[:, C32_MK1 : C32_MK1 + 1]
            bias_t = c32_sb[0:O, C32_BIAS : C32_BIAS + 1]

            # PSUM accumulator [O, 1024] (2 banks); its first use doubles as
            # the warm matmul that absorbs the c16 DMA semaphore into PE's
            # clock (walrus allows only one sync wait per instruction).
            acc = accp.tile([O, NCHUNK * CHUNK], F32)
            nc.tensor.matmul(
                acc[0:64, 0:128],
                cs16_sb[:, 0:64],
                cs16_sb[:, 0:128],
                start=True,
                stop=True,
            )
            nc.tensor.matmul(
                acc[0:64, 0:128],
                c16_sb[:, 0:64],
                c16_sb[:, 0:128],
                start=True,
                stop=True,
            )
            nc.tensor.matmul(
                acc[0:64, 0:128],
                c16_sb[:, GSPLIT * 3 * O : GSPLIT * 3 * O + 64],
                c16_sb[:, GSPLIT * 3 * O : GSPLIT * 3 * O + 128],
                start=True,
                stop=True,
            )
            # absorb input DMAs into DVE + ACT clocks
            ab = consts.tile([128, 2], F32, tag="absorb")
            nc.vector.tensor_copy(ab[:, 0:1], c32_sb[:, 0:1])
            nc.vector.tensor_copy(ab[0:D, 1:2], xt_sb[:, 0:1])
            nc.scalar.copy(ab[0:D, 1:2], xt_sb[:, 0:1])
            nc.scalar.copy(ab[:, 0:1], c32_sb[:, 0:1])

            # t = (x + 2) * (1/h); tc = clamp(t, 0, K-1); fp16 hi/lo split
            t_sb = consts.tile([D, BSH], F32)
            tc_sb = consts.tile([D, BSH], F32)
            nc.vector.tensor_scalar(
                t_sb[:], xt_sb[:], float(-X_MIN), float(1.0 / H), ALU.add, ALU.mult
            )
            nc.vector.tensor_scalar(
                tc_sb[:], t_sb[:], 0.0, float(K - 1), ALU.max, ALU.min
            )
            tc2 = consts.tile([128, BSH], F16)        # rows 0:64 hi, 64:128 lo
            tlo = consts.tile([D, BSH], F32)
            nc.vector.tensor_copy(tc2[0:D, :], tc_sb[:])
            nc.vector.tensor_tensor(tlo[:], tc_sb[:], tc2[0:D, :], ALU.subtract)
            nc.vector.tensor_copy(tc2[D:128, :], tlo[:])

            # Edge (extrapolation) fields, one per chunk:
            # rows 0:64 = min(t,0) -> hS[d,0]; rows 64:128 = max(t-63,0) -> hS[d,63]
            edges = []
            for c in range(NCHUNK):
                e = consts.tile([128, CHUNK], F16, tag=f"edge{c}")
                rows = t_sb[:, c * CHUNK : (c + 1) * CHUNK]
                nc.vector.tensor_scalar(e[0:64, :], rows, 0.0, None, ALU.min)
                nc.vector.tensor_scalar(
                    e[64:128, :], rows, float(-(K - 1)), 0.0, ALU.add, ALU.max
                )
                edges.append(e)

            ob_full = consts.tile([O, BSH], F32)

            for _rep in range(max(1, repeat)):
                for g in range(NPAIR):
                    # broadcast t of (d0,d1)=(2g,2g+1): hi+lo fp16 split matmul
                    tcb = tcbp.tile([128, NCHUNK * CHUNK], F32, tag="tcb")
                    for c in range(NCHUNK):
                        nc.tensor.matmul(
                            tcb[:, c * CHUNK : (c + 1) * CHUNK],
                            bsel_t(g),
                            tc2[:, c * CHUNK : (c + 1) * CHUNK],
                            start=True,
                            stop=True,
                        )
                    # hat half-fields: ap = relu(1-y), bp = relu(1+y), y = t-k
                    bp_ = work.tile([128, NCHUNK * CHUNK], F16, tag="bp_")
                    nc.scalar.activation(
                        bp_[:], tcb[:], AF.Relu, bias=mk1_t, scale=1.0
                    )
                    w = work.tile([128, NCHUNK * CHUNK], F16, tag="w")
                    if g % 4 == 1:
                        # DVE-only hat: w = max(min(2-bp, bp), 0)
                        r_ = work.tile([128, NCHUNK * CHUNK], F16, tag="r_")
                        nc.vector.tensor_scalar(
                            r_[:], bp_[:], -1.0, 2.0, ALU.mult, ALU.add
                        )
                        m1 = work.tile([128, NCHUNK * CHUNK], F16, tag="m1")
                        nc.vector.tensor_tensor(m1[:], r_[:], bp_[:], ALU.min)
                        nc.vector.tensor_scalar(w[:], m1[:], 0.0, None, ALU.max)
                    else:
                        ap_ = work.tile([128, NCHUNK * CHUNK], F16, tag="ap_")
                        nc.scalar.activation(
                            ap_[:], tcb[:], AF.Relu, bias=pk1_t, scale=-1.0
                        )
                        nc.vector.tensor_tensor(w[:], ap_[:], bp_[:], ALU.min)
                    w2 = work.tile([128, NCHUNK * CHUNK], F16, tag="w2")
                    nc.vector.tensor_tensor(w2[:], w[:], w[:], ALU.mult)
                    w3 = work.tile([128, NCHUNK * CHUNK], F16, tag="w3")
                    nc.vector.tensor_tensor(w3[:], w2[:], w[:], ALU.mult)
                    wb = work.tile([128, NCHUNK * CHUNK], F16, tag="wb")
                    if g % 8 == 7:
                        nc.vector.tensor_tensor(wb[:], w2[:], bp_[:], ALU.mult)
                    else:
                        nc.gpsimd.tensor_tensor(wb[:], w2[:], bp_[:], ALU.mult)
                    for c in range(NCHUNK):
                        sl = slice(c * CHUNK, (c + 1) * CHUNK)
                        out_sl = acc[:, sl]
                        nc.tensor.matmul(
                            out_sl, tab(C16_T1, g), w2[:, sl],
                            start=(g == 0), stop=False,
                        )
                        nc.tensor.matmul(
                            out_sl, tab(C16_T2, g), w3[:, sl],
                            start=False, stop=False,
                        )
                        nc.tensor.matmul(
                            out_sl, tab(C16_T3, g), wb[:, sl],
                            start=False, stop=False,
                        )
                for c in range(NCHUNK):
                    out_sl = acc[:, c * CHUNK : (c + 1) * CHUNK]
                    nc.tensor.matmul(
                        out_sl, etab_t, edges[c][:], start=False, stop=True
                    )
                    nc.vector.tensor_scalar(
                        ob_full[:, c * CHUNK : (c + 1) * CHUNK], out_sl, bias_t,
                        None, ALU.add,
                    )
                nc.sync.dma_start(outt[:], ob_full[:])

    _split_multiwaits(nc)
    return nc


def _split_multiwaits(nc):
    """walrus (neuronx-cc) allows one sync wait per instruction; move extra
    waits onto standalone NoOps inserted just before the offender."""
    cnt = 0
    for f in nc.m.functions:
        for blk in f.blocks:
            out = []
            changed = False
            for ins in blk.instructions:
                si = ins.sync_info
                if si is not None and len(si.on_wait) > 1:
                    waits = list(si.on_wait)
                    for w in waits[:-1]:
                        nop = mybir.InstNoOp(name=f"I-ws-{cnt}", ins=[], outs=[])
                        cnt += 1
                        nop.engine = ins.engine
                        nop.sync_info = type(si)(on_wait=[w], on_update=[])
                        out.append(nop)
                    ins.sync_info = type(si)(
                        on_wait=[waits[-1]], on_update=list(si.on_update)
                    )
                    changed = True
                out.append(ins)
            if changed:
                blk.instructions = out


def _host_tables(coeffs, bias):
    coeffs = np.ascontiguousarray(np.asarray(coeffs, dtype=np.float32))
    bias = np.asarray(bias, dtype=np.float32)
    slopes = _pchip_slopes_uniform(coeffs, H)          # [O, D, K]
    hs = (slopes * np.float32(H)).astype(np.float32)   # h * S

    ct = coeffs.transpose(1, 2, 0)                     # [D, K, O]
    st = hs.transpose(1, 2, 0)                         # [D, K, O]

    def pairs(a):                                      # [D,K,O] -> [128, 32*O]
        return np.ascontiguousarray(
            a.reshape(NPAIR, 2 * K, O).transpose(1, 0, 2).reshape(128, NPAIR * O)
        )

    c16 = np.zeros((128, CF16), dtype=np.float16)
    p1, p2, p3 = pairs(3.0 * ct - st), pairs(-2.0 * ct), pairs(st)
    for g in range(NPAIR):
        lo = g * 3 * O
        c16[:, lo : lo + O] = p1[:, g * O : (g + 1) * O]
        c16[:, lo + O : lo + 2 * O] = p2[:, g * O : (g + 1) * O]
        c16[:, lo + 2 * O : lo + 3 * O] = p3[:, g * O : (g + 1) * O]
    cs16 = np.zeros((128, CS16), dtype=np.float16)
    cs16[0:64, CS_ETAB : CS_ETAB + O] = st[:, 0, :]
    cs16[64:128, CS_ETAB : CS_ETAB + O] = st[:, K - 1, :]
    for g in range(NPAIR):
        base = CS_BSEL + g * 128
        cs16[2 * g, base : base + 64] = 1.0            # hi row d0 -> parts 0:64
        cs16[2 * g + 1, base + 64 : base + 128] = 1.0  # hi row d1 -> parts 64:128
        cs16[64 + 2 * g, base : base + 64] = 1.0       # lo row d0
        cs16[64 + 2 * g + 1, base + 64 : base + 128] = 1.0

    c32 = np.zeros((128, CF32), dtype=np.float32)
    kk = np.arange(128, dtype=np.float32) % K
    c32[:, C32_PK1] = 1.0 + kk
    c32[:, C32_MK1] = 1.0 - kk
    c32[0:O, C32_BIAS] = bias
    return c16, cs16, c32


def kernel(x, coeffs, bias):
    global LAST_EXEC_NS
    x = np.asarray(x, dtype=np.float32)
    c16, cs16, c32 = _host_tables(coeffs, bias)

    in_maps = []
    for r in range(NCORES):
        xc = x[r * BSH : (r + 1) * BSH, :]             # [1024, 64]
        in_maps.append(
            {"xt": np.ascontiguousarray(xc.T), "c16": c16, "cs16": cs16, "c32": c32}
        )

    nc = _build_kernel()
    res = run_bass_kernel_spmd(nc, in_maps, list(range(NCORES)), trace=TRACE)
    LAST_EXEC_NS = getattr(res, "exec_time_ns", None)

    out = np.empty((B, O), dtype=np.float32)
    for r in range(NCORES):
        out_t = res.results[r]["outt"]                 # [O, 1024]
        out[r * BSH : (r + 1) * BSH, :] = np.asarray(out_t).T
    return out


if __name__ == "__main__":
    rng = np.random.default_rng(0)
    x = rng.standard_normal((B, D)).astype(np.float32)
    coeffs = (0.01 * rng.standard_normal((O, D, K))).astype(np.float32)
    bias = np.zeros((O,), dtype=np.float32)
    out = kernel(x, coeffs, bias)
    print("out", out.shape, out.dtype, float(np.abs(out).mean()))



# revision 21
# speedup vs baseline: 1.7228x; 1.7228x over previous
"""Trainium2 Bass kernel for a bare KAN layer (PCHIP spline mixing).

Math: out[b, o] = sum_d f_{o,d}(x[b,d]) + bias[o], where f_{o,d} is the PCHIP
cubic interpolant of coeffs[o,d,:] on K=64 uniform knots over [-2, 2], with
linear extrapolation outside.

Device strategy (per core, data-parallel over batch):
  Segment-power telescoping basis. With t = (x - X_MIN)/h and
  u_s = clamp(t - s, 0, 1) for segments s = 0..K-2:

      f(t) = f(0) + sum_s g_s(u_s),   g_s(u) = b_s u + c_s u^2 + d_s u^3

  because each g_s vanishes at u=0 and the u=1 plateaus telescope to
  f(floor) - f(0) exactly; linear extrapolation outside the domain is the
  extra  -hS_0*relu(-t) + hS_{K-1}*relu(t-(K-1))  term.

  Per group of 128 rows (64 dims x 2 segments) the fields are built with
  four engine ops -- y = ACT Identity(t - s) (fp32->fp16), u = DVE
  clamp(y,0,1) (4x mode), u2 = DVE u*u, u3 = Pool u2*u -- and contracted
  against fp16 tables b/c/d in PSUM. t is replicated [t;t] host-side, so
  there is no per-group broadcast matmul.

Self-contained: hardcodes shapes B=8192, D=64, K=64, O=64, 8 cores.
"""

import sys

import numpy as np

sys.path.insert(0, "/opt/trn_rl_repo")

from concourse import bass, mybir  # noqa: E402
from concourse.bass_utils import run_bass_kernel_spmd  # noqa: E402
from concourse.tile import TileContext  # noqa: E402

F32 = mybir.dt.float32
F16 = mybir.dt.float16
ALU = mybir.AluOpType
AF = mybir.ActivationFunctionType

B, D, K, O = 8192, 64, 64, 64
NCORES = 8
BSH = B // NCORES          # 1024 batch rows per core
NCHUNK = 2                 # 512-column matmul chunks
CHUNK = BSH // NCHUNK      # 512
NS = K - 1                 # 63 segments
NGRP = 32                  # groups of 2 segments (last half padded)
X_MIN, X_MAX = -2.0, 2.0
H = (X_MAX - X_MIN) / (K - 1)

CTB = NGRP * 3 * O         # 6144 table cols: per group [b | c | d] x O
TB_SPLIT = 8 * 3 * O       # first-chunk table DMA (groups 0..7)

# sb const tensor [128, 34] fp32: cols 0..31 group biases (-s per partition),
# col 32 = -(K-1) edge-hi bias, col 33 = 0.0 edge-lo bias
SB_EHI = 32
SB_ELO = 33
CSB = 34

WARM_N = 9                 # PE p-state warm matmuls bridging the DMA wait
EDGE_AT = 8                # group index after which edge fields are built
U3_DVE = set()             # groups whose u3 runs on DVE instead of Pool
U2_POOL = {3, 9, 15, 21, 27}  # groups whose u2 runs on Pool to unload DVE

TRACE = False
LAST_EXEC_NS = None


def _pchip_slopes_uniform(y, h):
    """numpy float32 port of reference._pchip_slopes_uniform. y: [..., K]."""
    y = y.astype(np.float32)
    delta = ((y[..., 1:] - y[..., :-1]) / np.float32(h)).astype(np.float32)
    dp, dn = delta[..., :-1], delta[..., 1:]
    same_sign = dp * dn > 0
    d_mid = np.where(
        same_sign, (2.0 * dp * dn / (dp + dn + np.float32(1e-12))), np.float32(0.0)
    ).astype(np.float32)

    def _fix_endpoint(d_end, delta0, delta1):
        d_end = np.where(d_end * delta0 <= 0, np.float32(0.0), d_end)
        d_end = np.where(
            (delta0 * delta1 < 0) & (np.abs(d_end) > 3.0 * np.abs(delta0)),
            (3.0 * delta0).astype(np.float32),
            d_end,
        )
        return d_end.astype(np.float32)

    d0 = _fix_endpoint(
        ((3.0 * delta[..., 0] - delta[..., 1]) / 2.0).astype(np.float32),
        delta[..., 0],
        delta[..., 1],
    )
    dN = _fix_endpoint(
        ((3.0 * delta[..., -1] - delta[..., -2]) / 2.0).astype(np.float32),
        delta[..., -1],
        delta[..., -2],
    )
    return np.concatenate([d0[..., None], d_mid, dN[..., None]], axis=-1)


def _build_kernel():
    nc = bass.Bass()

    t2 = nc.declare_dram_parameter("t2", [128, BSH], F32, isOutput=False)
    tb = nc.declare_dram_parameter("tb", [128, CTB], F16, isOutput=False)
    etab = nc.declare_dram_parameter("etab", [128, O], F16, isOutput=False)
    sb = nc.declare_dram_parameter("sb", [128, CSB], F32, isOutput=False)
    k0 = nc.declare_dram_parameter("k0", [O, 1], F32, isOutput=False)
    outt = nc.declare_dram_parameter("outt", [O, BSH], F32, isOutput=True)

    with TileContext(nc) as tc:
        with (
            tc.tile_pool(name="consts", bufs=1) as consts,
            tc.tile_pool(name="work", bufs=3) as work,
            tc.tile_pool(name="accp", bufs=1, space="PSUM") as accp,
        ):
            t2_sb = consts.tile([128, BSH], F32)
            tb_sb = consts.tile([128, CTB], F16)
            etab_sb = consts.tile([128, O], F16)
            sb_sb = consts.tile([128, CSB], F32)
            k0_sb = consts.tile([O, 1], F32)
            # sb + first table chunk serially on the SP queue; t2 halves on
            # the DVE/ACT queues in parallel so group 0 starts ~1us earlier
            nc.sync.dma_start(sb_sb[:], sb[:])
            nc.scalar.dma_start(t2_sb[:, 0:CHUNK], t2[:, 0:CHUNK])
            nc.gpsimd.dma_start(t2_sb[:, CHUNK:], t2[:, CHUNK:])
            nc.sync.dma_start(tb_sb[:, :TB_SPLIT], tb[:, :TB_SPLIT])
            nc.sync.dma_start(tb_sb[:, TB_SPLIT:], tb[:, TB_SPLIT:])
            nc.sync.dma_start(etab_sb[:], etab[:])
            nc.sync.dma_start(k0_sb[:], k0[:])

            def grp_tab(j, f):
                lo = j * 3 * O + f * O
                return tb_sb[:, lo : lo + O]

            # PSUM accumulator [O, 1024] (2 banks). Warm matmuls keep the PE
            # p-state ramp going from t=0 on a memset tile; results are
            # discarded by the start=True restarts below.
            # one PSUM tile per 512-col chunk so chunk 0's output path does
            # not serialize behind chunk 1's accumulation (tile-granularity
            # dependency tracking)
            acc0 = accp.tile([O, CHUNK], F32)
            acc1 = accp.tile([O, CHUNK], F32)
            accs = [acc0, acc1]
            warm = consts.tile([128, 512], F16, tag="warm")
            nc.vector.memset(warm[:], 0.0)
            # preload the activation-function table before t2 arrives so the
            # first y doesn't pay the 1283ns table load (separate output tile
            # so the warm matmuls below don't serialize behind it)
            dummy = consts.tile([1, 1], F16, tag="dummy")
            nc.scalar.activation(dummy[:], warm[0:1, 0:1], AF.Identity)
            for _ in range(WARM_N):
                nc.tensor.matmul(
                    acc0[0:64, 0:512],
                    warm[:, 0:64],
                    warm[:, 0:512],
                    start=True,
                    stop=True,
                )

            edges = consts.tile([128, BSH], F16, tag="edges")
            obs = []
            for q in range(4):
                ob_q = consts.tile([O, CHUNK // 2], F32, tag=f"ob{q}", name=f"ob{q}")
                obs.append(ob_q)

            for j in range(NGRP):
                y = work.tile([128, BSH], F16, tag="y")
                u = work.tile([128, BSH], F16, tag="u")
                u2 = work.tile([128, BSH], F16, tag="u2")
                u3 = work.tile([128, BSH], F16, tag="u3")
                # group 0 is built in column halves so its first matmuls only
                # wait on the first half of the t2 DMA
                halves = (
                    [slice(0, CHUNK), slice(CHUNK, BSH)] if j == 0 else [slice(0, BSH)]
                )
                for h in halves:
                    nc.scalar.activation(
                        y[:, h], t2_sb[:, h], AF.Identity,
                        bias=sb_sb[:, j : j + 1], scale=1.0,
                    )
                    nc.vector.tensor_scalar(
                        u[:, h], y[:, h], 0.0, 1.0, ALU.max, ALU.min
                    )
                    if j in U2_POOL:
                        nc.gpsimd.tensor_tensor(u2[:, h], u[:, h], u[:, h], ALU.mult)
                    else:
                        nc.vector.tensor_tensor(u2[:, h], u[:, h], u[:, h], ALU.mult)
                    if j in U3_DVE:
                        nc.vector.tensor_tensor(u3[:, h], u2[:, h], u[:, h], ALU.mult)
                    else:
                        nc.gpsimd.tensor_tensor(u3[:, h], u2[:, h], u[:, h], ALU.mult)

                if j == EDGE_AT:
                    # edge (extrapolation) fields, computed mid-stream where
                    # DVE has slack: rows 0:64 = relu(-t) -> -hS[d,0],
                    # rows 64:128 = relu(t-63) -> hS[d,63]
                    nc.vector.tensor_scalar(
                        edges[0:64, :], t2_sb[0:64, :], -1.0, 0.0,
                        ALU.mult, ALU.max,
                    )
                    nc.vector.tensor_scalar(
                        edges[64:128, :], t2_sb[64:128, :], float(-(K - 1)),
                        0.0, ALU.add, ALU.max,
                    )

                last = j == NGRP - 1
                for c in range(NCHUNK):
                    sl = slice(c * CHUNK, (c + 1) * CHUNK)
                    nc.tensor.matmul(
                        accs[c][:], grp_tab(j, 0), u[:, sl],
                        start=(j == 0), stop=False,
                    )
                    nc.tensor.matmul(
                        accs[c][:], grp_tab(j, 1), u2[:, sl],
                        start=False, stop=False,
                    )
                    nc.tensor.matmul(
                        accs[c][:], grp_tab(j, 2), u3[:, sl],
                        start=False, stop=False,
                    )
                    if last:
                        nc.tensor.matmul(
                            accs[c][:], etab_sb[:], edges[:, sl],
                            start=False, stop=True,
                        )

            # bias/const add + DMA out in 256-col pieces, after ALL matmuls
            # (acc is one tile: an early read would add a write-after-read
            # stall on the remaining accumulation). Separate ob tiles so the
            # four pieces don't serialize; DMAs spread across queues.
            dma_eng = [nc.sync, nc.gpsimd, nc.sync, nc.scalar]
            for q in range(4):
                qsl = slice(q * (CHUNK // 2), (q + 1) * (CHUNK // 2))
                asl = slice((q % 2) * (CHUNK // 2), (q % 2 + 1) * (CHUNK // 2))
                if q % 2 == 0:
                    nc.scalar.activation(
                        obs[q][:], accs[q // 2][:, asl], AF.Identity,
                        bias=k0_sb[:, 0:1], scale=1.0,
                    )
                else:
                    nc.vector.tensor_scalar(
                        obs[q][:], accs[q // 2][:, asl], k0_sb[:, 0:1], None, ALU.add
                    )
                dma_eng[q].dma_start(outt[:, qsl], obs[q][:])

    _split_multiwaits(nc)
    return nc


def _split_multiwaits(nc):
    """walrus (neuronx-cc) allows one sync wait per instruction; move extra
    waits onto standalone NoOps inserted just before the offender."""
    cnt = 0
    for f in nc.m.functions:
        for blk in f.blocks:
            out = []
            changed = False
            for ins in blk.instructions:
                si = ins.sync_info
                if si is not None and len(si.on_wait) > 1:
                    waits = list(si.on_wait)
                    for w in waits[:-1]:
                        nop = mybir.InstNoOp(name=f"I-ws-{cnt}", ins=[], outs=[])
                        cnt += 1
                        nop.engine = ins.engine
                        nop.sync_info = type(si)(on_wait=[w], on_update=[])
                        out.append(nop)
                    ins.sync_info = type(si)(
                        on_wait=[waits[-1]], on_update=list(si.on_update)
                    )
                    changed = True
                out.append(ins)
            if changed:
                blk.instructions = out


def _host_tables(coeffs, bias):
    coeffs = np.ascontiguousarray(np.asarray(coeffs, dtype=np.float32))
    bias = np.asarray(bias, dtype=np.float32)
    slopes = _pchip_slopes_uniform(coeffs, H)          # [O, D, K]
    hs = (slopes * np.float32(H)).astype(np.float32)   # h * S

    C = coeffs
    dC = C[..., 1:] - C[..., :-1]                      # [O, D, NS]
    c = (3.0 * dC - 2.0 * hs[..., :-1] - hs[..., 1:]).astype(np.float32)
    d = (-2.0 * dC + hs[..., :-1] + hs[..., 1:]).astype(np.float32)
    c16 = c.astype(np.float16)
    d16 = d.astype(np.float16)
    # compensate b so the u=1 plateau sum b+c+d telescopes to dC as exactly
    # as fp16 allows
    b16 = (dC - c16.astype(np.float32) - d16.astype(np.float32)).astype(np.float16)

    tb = np.zeros((128, CTB), dtype=np.float16)
    tabs = (b16, c16, d16)
    for j in range(NGRP):
        for half in range(2):
            s = 2 * j + half
            if s >= NS:
                continue
            rows = slice(half * 64, (half + 1) * 64)
            for f in range(3):
                lo = j * 3 * O + f * O
                # rows = dims, cols = o
                tb[rows, lo : lo + O] = tabs[f][:, :, s].T

    etab = np.zeros((128, O), dtype=np.float16)
    etab[0:64, :] = -hs[:, :, 0].T
    etab[64:128, :] = hs[:, :, K - 1].T

    sb = np.zeros((128, CSB), dtype=np.float32)
    for j in range(NGRP):
        sb[0:64, j] = -(2 * j)
        sb[64:128, j] = -(2 * j + 1)
    sb[:, SB_EHI] = -(K - 1)
    sb[:, SB_ELO] = 0.0

    k0 = (C[..., 0].sum(axis=1) + bias).astype(np.float32).reshape(O, 1)
    return tb, etab, sb, k0


def kernel(x, coeffs, bias):
    global LAST_EXEC_NS
    x = np.asarray(x, dtype=np.float32)
    tb, etab, sb, k0 = _host_tables(coeffs, bias)

    in_maps = []
    for r in range(NCORES):
        xc = x[r * BSH : (r + 1) * BSH, :]             # [1024, 64]
        t = ((xc.T - np.float32(X_MIN)) * np.float32(1.0 / H)).astype(np.float32)
        t2 = np.ascontiguousarray(np.concatenate([t, t], axis=0))  # [128, 1024]
        in_maps.append(
            {"t2": t2, "tb": tb, "etab": etab, "sb": sb, "k0": k0}
        )

    nc = _build_kernel()
    res = run_bass_kernel_spmd(nc, in_maps, list(range(NCORES)), trace=TRACE)
    LAST_EXEC_NS = getattr(res, "exec_time_ns", None)

    out = np.empty((B, O), dtype=np.float32)
    for r in range(NCORES):
        out_t = res.results[r]["outt"]                 # [O, 1024]
        out[r * BSH : (r + 1) * BSH, :] = np.asarray(out_t).T
    return out


if __name__ == "__main__":
    rng = np.random.default_rng(0)
    x = rng.standard_normal((B, D)).astype(np.float32)
    coeffs = (0.01 * rng.standard_normal((O, D, K))).astype(np.float32)
    bias = np.zeros((O,), dtype=np.float32)
    out = kernel(x, coeffs, bias)
    print("out", out.shape, out.dtype, float(np.abs(out).mean()))


# revision 30
# speedup vs baseline: 1.9159x; 1.1121x over previous
"""Trainium2 Bass kernel for a bare KAN layer (PCHIP spline mixing).

Math: out[b, o] = sum_d f_{o,d}(x[b,d]) + bias[o], where f_{o,d} is the PCHIP
cubic interpolant of coeffs[o,d,:] on K=64 uniform knots over [-2, 2], with
linear extrapolation outside.

Device strategy (per core, data-parallel over batch):
  Segment-power telescoping basis. With t = (x - X_MIN)/h and
  u_s = clamp(t - s, 0, 1) for segments s = 0..K-2:

      f(t) = f(0) + sum_s g_s(u_s),   g_s(u) = b_s u + c_s u^2 + d_s u^3

  because each g_s vanishes at u=0 and the u=1 plateaus telescope to
  f(floor) - f(0) exactly; linear extrapolation outside the domain is the
  extra  -hS_0*relu(-t) + hS_{K-1}*relu(t-(K-1))  term.

  Per group of 128 rows (64 dims x 2 segments) the fields are built with
  four engine ops -- y = ACT Identity(t - s) (fp32->fp16), u = DVE
  clamp(y,0,1) (4x mode), u2 = DVE u*u, u3 = Pool u2*u -- and contracted
  against fp16 tables b/c/d in PSUM. t is replicated [t;t] host-side, so
  there is no per-group broadcast matmul.

Self-contained: hardcodes shapes B=8192, D=64, K=64, O=64, 8 cores.
"""

import sys

import numpy as np

sys.path.insert(0, "/opt/trn_rl_repo")

from concourse import bass, mybir  # noqa: E402
from concourse.bass_utils import run_bass_kernel_spmd  # noqa: E402
from concourse.tile import TileContext  # noqa: E402

F32 = mybir.dt.float32
F16 = mybir.dt.float16
F8 = mybir.dt.float8e4
ALU = mybir.AluOpType
AF = mybir.ActivationFunctionType
PM = mybir.MatmulPerfMode

B, D, K, O = 8192, 64, 64, 64
NCORES = 8
BSH = B // NCORES          # 1024 batch rows per core
NCHUNK = 2                 # 512-column matmul chunks
CHUNK = BSH // NCHUNK      # 512
NS = K - 1                 # 63 segments
NGRP = 32                  # groups of 2 segments (last half padded)
X_MIN, X_MAX = -2.0, 2.0
H = (X_MAX - X_MIN) / (K - 1)

CTB = NGRP * 3 * O         # 6144 table cols: per group [b | c | d] x O
TB_SPLIT = 8 * 3 * O       # first-chunk table DMA (groups 0..7)

# sb const tensor [128, 34] fp32: cols 0..31 group biases (-s per partition),
# col 32 = -(K-1) edge-hi bias, col 33 = 0.0 edge-lo bias
SB_EHI = 32
SB_ELO = 33
CSB = 34

WARM_N = 9                 # PE p-state warm matmuls bridging the DMA wait
EDGE_AT = 8                # group index after which edge fields are built
U3_DVE = {2, 6, 10, 14, 18, 22, 26, 30}  # fp16 groups whose u3 runs on DVE
U2_POOL = set()            # fp16 groups whose u2 runs on Pool to unload DVE
# Groups evaluated via fp8-e4m3 DoubleRow: fields q=u(u-1), r=q*u (zero on
# both plateaus, so fp8 tables only touch the locally-active segment) with
# tables (c+d, d); the u-field stays fp16 with table dC. Interleaved with
# fp16 groups so Pool's two fp8 writes per DR group pipeline against ACT's
# y cadence.
DR_GROUPS = frozenset(range(1, 32, 2))
NDR = len(DR_GROUPS)

TRACE = False
LAST_EXEC_NS = None


def _pchip_slopes_uniform(y, h):
    """numpy float32 port of reference._pchip_slopes_uniform. y: [..., K]."""
    y = y.astype(np.float32)
    delta = ((y[..., 1:] - y[..., :-1]) / np.float32(h)).astype(np.float32)
    dp, dn = delta[..., :-1], delta[..., 1:]
    same_sign = dp * dn > 0
    d_mid = np.where(
        same_sign, (2.0 * dp * dn / (dp + dn + np.float32(1e-12))), np.float32(0.0)
    ).astype(np.float32)

    def _fix_endpoint(d_end, delta0, delta1):
        d_end = np.where(d_end * delta0 <= 0, np.float32(0.0), d_end)
        d_end = np.where(
            (delta0 * delta1 < 0) & (np.abs(d_end) > 3.0 * np.abs(delta0)),
            (3.0 * delta0).astype(np.float32),
            d_end,
        )
        return d_end.astype(np.float32)

    d0 = _fix_endpoint(
        ((3.0 * delta[..., 0] - delta[..., 1]) / 2.0).astype(np.float32),
        delta[..., 0],
        delta[..., 1],
    )
    dN = _fix_endpoint(
        ((3.0 * delta[..., -1] - delta[..., -2]) / 2.0).astype(np.float32),
        delta[..., -1],
        delta[..., -2],
    )
    return np.concatenate([d0[..., None], d_mid, dN[..., None]], axis=-1)


def _build_kernel():
    nc = bass.Bass()

    t2 = nc.declare_dram_parameter("t2", [128, BSH], F32, isOutput=False)
    tb = nc.declare_dram_parameter("tb", [128, CTB], F16, isOutput=False)
    tb8 = nc.declare_dram_parameter("tb8", [128, 2, NDR * O], F8, isOutput=False)
    etab = nc.declare_dram_parameter("etab", [128, O], F16, isOutput=False)
    sb = nc.declare_dram_parameter("sb", [128, CSB], F32, isOutput=False)
    k0 = nc.declare_dram_parameter("k0", [O, 1], F32, isOutput=False)
    outt = nc.declare_dram_parameter("outt", [O, BSH], F32, isOutput=True)

    with TileContext(nc) as tc:
        with (
            tc.tile_pool(name="consts", bufs=1) as consts,
            tc.tile_pool(name="work", bufs=3) as work,
            tc.tile_pool(name="accp", bufs=1, space="PSUM") as accp,
        ):
            t2_sb = consts.tile([128, BSH], F32)
            tb_sb = consts.tile([128, CTB], F16)
            tb8_sb = consts.tile([128, 2, NDR * O], F8)
            etab_sb = consts.tile([128, O], F16)
            sb_sb = consts.tile([128, CSB], F32)
            k0_sb = consts.tile([O, 1], F32)
            # sb + first table chunk serially on the SP queue; t2 halves on
            # the DVE/ACT queues in parallel so group 0 starts ~1us earlier
            nc.sync.dma_start(sb_sb[:], sb[:])
            nc.scalar.dma_start(t2_sb[:, 0:CHUNK], t2[:, 0:CHUNK])
            nc.gpsimd.dma_start(t2_sb[:, CHUNK:], t2[:, CHUNK:])
            nc.sync.dma_start(tb_sb[:, :TB_SPLIT], tb[:, :TB_SPLIT])
            nc.sync.dma_start(tb8_sb[:], tb8[:])
            nc.sync.dma_start(tb_sb[:, TB_SPLIT:], tb[:, TB_SPLIT:])
            nc.sync.dma_start(etab_sb[:], etab[:])
            nc.sync.dma_start(k0_sb[:], k0[:])

            dr_list = sorted(DR_GROUPS)

            def grp_tab(j, f):
                lo = j * 3 * O + f * O
                return tb_sb[:, lo : lo + O]

            def dr_tab(j):
                gi = dr_list.index(j)
                return tb8_sb[:, :, gi * O : (gi + 1) * O]

            # PSUM accumulator [O, 1024] (2 banks). Warm matmuls keep the PE
            # p-state ramp going from t=0 on a memset tile; results are
            # discarded by the start=True restarts below.
            # one PSUM tile per 512-col chunk so chunk 0's output path does
            # not serialize behind chunk 1's accumulation (tile-granularity
            # dependency tracking)
            acc0 = accp.tile([O, CHUNK], F32)
            acc1 = accp.tile([O, CHUNK], F32)
            accs = [acc0, acc1]
            warm = consts.tile([128, 512], F16, tag="warm")
            nc.vector.memset(warm[:], 0.0)
            # preload the activation-function table before t2 arrives so the
            # first y doesn't pay the 1283ns table load (separate output tile
            # so the warm matmuls below don't serialize behind it)
            dummy = consts.tile([1, 1], F16, tag="dummy")
            nc.scalar.activation(dummy[:], warm[0:1, 0:1], AF.Identity)
            for _ in range(WARM_N):
                nc.tensor.matmul(
                    acc0[0:64, 0:512],
                    warm[:, 0:64],
                    warm[:, 0:512],
                    start=True,
                    stop=True,
                )

            edges = consts.tile([128, BSH], F16, tag="edges")
            obs = []
            for q in range(4):
                ob_q = consts.tile([O, CHUNK // 2], F32, tag=f"ob{q}", name=f"ob{q}")
                obs.append(ob_q)

            for j in range(NGRP):
                is_dr = j in DR_GROUPS
                y = work.tile([128, BSH], F16, tag="y")
                u = work.tile([128, BSH], F16, tag="u")
                if is_dr:
                    qa = work.tile([128, BSH], F16, tag="qa")
                    qr = work.tile([128, 2, BSH], F8, tag="qr")
                else:
                    u2 = work.tile([128, BSH], F16, tag="u2")
                    u3 = work.tile([128, BSH], F16, tag="u3")
                # group 0 is built in column halves so its first matmuls only
                # wait on the first half of the t2 DMA
                halves = (
                    [slice(0, CHUNK), slice(CHUNK, BSH)] if j == 0 else [slice(0, BSH)]
                )
                for h in halves:
                    nc.scalar.activation(
                        y[:, h], t2_sb[:, h], AF.Identity,
                        bias=sb_sb[:, j : j + 1], scale=1.0,
                    )
                    nc.vector.tensor_scalar(
                        u[:, h], y[:, h], 0.0, 1.0, ALU.max, ALU.min
                    )
                    if is_dr:
                        nc.vector.tensor_scalar(qa[:, h], u[:, h], -1.0, None, ALU.add)
                        nc.gpsimd.tensor_tensor(
                            qr[:, 0, h], u[:, h], qa[:, h], ALU.mult
                        )
                        nc.gpsimd.tensor_tensor(
                            qr[:, 1, h], qr[:, 0, h], u[:, h], ALU.mult
                        )
                    elif j in U2_POOL:
                        nc.gpsimd.tensor_tensor(u2[:, h], u[:, h], u[:, h], ALU.mult)
                        nc.gpsimd.tensor_tensor(u3[:, h], u2[:, h], u[:, h], ALU.mult)
                    else:
                        nc.vector.tensor_tensor(u2[:, h], u[:, h], u[:, h], ALU.mult)
                        if j in U3_DVE:
                            nc.vector.tensor_tensor(
                                u3[:, h], u2[:, h], u[:, h], ALU.mult
                            )
                        else:
                            nc.gpsimd.tensor_tensor(
                                u3[:, h], u2[:, h], u[:, h], ALU.mult
                            )

                if j == EDGE_AT:
                    # edge (extrapolation) fields, computed mid-stream where
                    # DVE has slack: rows 0:64 = relu(-t) -> -hS[d,0],
                    # rows 64:128 = relu(t-63) -> hS[d,63]
                    nc.vector.tensor_scalar(
                        edges[0:64, :], t2_sb[0:64, :], -1.0, 0.0,
                        ALU.mult, ALU.max,
                    )
                    nc.vector.tensor_scalar(
                        edges[64:128, :], t2_sb[64:128, :], float(-(K - 1)),
                        0.0, ALU.add, ALU.max,
                    )

                last = j == NGRP - 1
                for c in range(NCHUNK):
                    sl = slice(c * CHUNK, (c + 1) * CHUNK)
                    nc.tensor.matmul(
                        accs[c][:], grp_tab(j, 0), u[:, sl],
                        start=(j == 0), stop=False,
                    )
                    if is_dr:
                        nc.tensor.matmul(
                            accs[c][:], dr_tab(j), qr[:, :, sl],
                            start=False, stop=False, perf_mode=PM.DoubleRow,
                        )
                    else:
                        nc.tensor.matmul(
                            accs[c][:], grp_tab(j, 1), u2[:, sl],
                            start=False, stop=False,
                        )
                        nc.tensor.matmul(
                            accs[c][:], grp_tab(j, 2), u3[:, sl],
                            start=False, stop=False,
                        )
                    if last:
                        nc.tensor.matmul(
                            accs[c][:], etab_sb[:], edges[:, sl],
                            start=False, stop=True,
                        )

            # bias/const add + DMA out in 256-col pieces, after ALL matmuls
            # (acc is one tile: an early read would add a write-after-read
            # stall on the remaining accumulation). Separate ob tiles so the
            # four pieces don't serialize; DMAs spread across queues.
            dma_eng = [nc.sync, nc.gpsimd, nc.sync, nc.scalar]
            for q in range(4):
                qsl = slice(q * (CHUNK // 2), (q + 1) * (CHUNK // 2))
                asl = slice((q % 2) * (CHUNK // 2), (q % 2 + 1) * (CHUNK // 2))
                if q % 2 == 0:
                    nc.scalar.activation(
                        obs[q][:], accs[q // 2][:, asl], AF.Identity,
                        bias=k0_sb[:, 0:1], scale=1.0,
                    )
                else:
                    nc.vector.tensor_scalar(
                        obs[q][:], accs[q // 2][:, asl], k0_sb[:, 0:1], None, ALU.add
                    )
                dma_eng[q].dma_start(outt[:, qsl], obs[q][:])

    _split_multiwaits(nc)
    return nc


def _split_multiwaits(nc):
    """walrus (neuronx-cc) allows one sync wait per instruction; move extra
    waits onto standalone NoOps inserted just before the offender."""
    cnt = 0
    for f in nc.m.functions:
        for blk in f.blocks:
            out = []
            changed = False
            for ins in blk.instructions:
                si = ins.sync_info
                if si is not None and len(si.on_wait) > 1:
                    waits = list(si.on_wait)
                    for w in waits[:-1]:
                        nop = mybir.InstNoOp(name=f"I-ws-{cnt}", ins=[], outs=[])
                        cnt += 1
                        nop.engine = ins.engine
                        nop.sync_info = type(si)(on_wait=[w], on_update=[])
                        out.append(nop)
                    ins.sync_info = type(si)(
                        on_wait=[waits[-1]], on_update=list(si.on_update)
                    )
                    changed = True
                out.append(ins)
            if changed:
                blk.instructions = out


def _host_tables(coeffs, bias):
    coeffs = np.ascontiguousarray(np.asarray(coeffs, dtype=np.float32))
    bias = np.asarray(bias, dtype=np.float32)
    slopes = _pchip_slopes_uniform(coeffs, H)          # [O, D, K]
    hs = (slopes * np.float32(H)).astype(np.float32)   # h * S

    C = coeffs
    dC = C[..., 1:] - C[..., :-1]                      # [O, D, NS]
    c = (3.0 * dC - 2.0 * hs[..., :-1] - hs[..., 1:]).astype(np.float32)
    d = (-2.0 * dC + hs[..., :-1] + hs[..., 1:]).astype(np.float32)
    c16 = c.astype(np.float16)
    d16 = d.astype(np.float16)
    # compensate b so the u=1 plateau sum b+c+d telescopes to dC as exactly
    # as fp16 allows
    b16 = (dC - c16.astype(np.float32) - d16.astype(np.float32)).astype(np.float16)

    from ml_dtypes import float8_e4m3fn as E4M3

    tb = np.zeros((128, CTB), dtype=np.float16)
    tb8v = np.zeros((128, 2, NDR * O), dtype=np.float32)
    dr_list = sorted(DR_GROUPS)
    tabs = (b16, c16, d16)
    for j in range(NGRP):
        is_dr = j in DR_GROUPS
        for half in range(2):
            s = 2 * j + half
            if s >= NS:
                continue
            rows = slice(half * 64, (half + 1) * 64)
            if is_dr:
                # u-field table = dC (plateau-exact); q,r tables in fp8
                gi = dr_list.index(j)
                lo = j * 3 * O
                tb[rows, lo : lo + O] = dC[:, :, s].T.astype(np.float16)
                tb8v[rows, 0, gi * O : (gi + 1) * O] = (c + d)[:, :, s].T
                tb8v[rows, 1, gi * O : (gi + 1) * O] = d[:, :, s].T
            else:
                for f in range(3):
                    lo = j * 3 * O + f * O
                    # rows = dims, cols = o
                    tb[rows, lo : lo + O] = tabs[f][:, :, s].T
    tb8 = tb8v.astype(E4M3)

    etab = np.zeros((128, O), dtype=np.float16)
    etab[0:64, :] = -hs[:, :, 0].T
    etab[64:128, :] = hs[:, :, K - 1].T

    sb = np.zeros((128, CSB), dtype=np.float32)
    for j in range(NGRP):
        sb[0:64, j] = -(2 * j)
        sb[64:128, j] = -(2 * j + 1)
    sb[:, SB_EHI] = -(K - 1)
    sb[:, SB_ELO] = 0.0

    k0 = (C[..., 0].sum(axis=1) + bias).astype(np.float32).reshape(O, 1)
    return tb, tb8, etab, sb, k0


def kernel(x, coeffs, bias):
    global LAST_EXEC_NS
    x = np.asarray(x, dtype=np.float32)
    tb, tb8, etab, sb, k0 = _host_tables(coeffs, bias)

    in_maps = []
    for r in range(NCORES):
        xc = x[r * BSH : (r + 1) * BSH, :]             # [1024, 64]
        t = ((xc.T - np.float32(X_MIN)) * np.float32(1.0 / H)).astype(np.float32)
        t2 = np.ascontiguousarray(np.concatenate([t, t], axis=0))  # [128, 1024]
        in_maps.append(
            {"t2": t2, "tb": tb, "tb8": tb8, "etab": etab, "sb": sb, "k0": k0}
        )

    nc = _build_kernel()
    res = run_bass_kernel_spmd(nc, in_maps, list(range(NCORES)), trace=TRACE)
    LAST_EXEC_NS = getattr(res, "exec_time_ns", None)

    out = np.empty((B, O), dtype=np.float32)
    for r in range(NCORES):
        out_t = res.results[r]["outt"]                 # [O, 1024]
        out[r * BSH : (r + 1) * BSH, :] = np.asarray(out_t).T
    return out


if __name__ == "__main__":
    rng = np.random.default_rng(0)
    x = rng.standard_normal((B, D)).astype(np.float32)
    coeffs = (0.01 * rng.standard_normal((O, D, K))).astype(np.float32)
    bias = np.zeros((O,), dtype=np.float32)
    out = kernel(x, coeffs, bias)
    print("out", out.shape, out.dtype, float(np.abs(out).mean()))


# revision 40
# speedup vs baseline: 1.9694x; 1.0279x over previous
"""Trainium2 Bass kernel for a bare KAN layer (PCHIP spline mixing).

Math: out[b, o] = sum_d f_{o,d}(x[b,d]) + bias[o], where f_{o,d} is the PCHIP
cubic interpolant of coeffs[o,d,:] on K=64 uniform knots over [-2, 2], with
linear extrapolation outside.

Device strategy (per core, data-parallel over batch):
  Segment-power telescoping basis. With t = (x - X_MIN)/h and
  u_s = clamp(t - s, 0, 1) for segments s = 0..K-2:

      f(t) = f(0) + sum_s g_s(u_s),   g_s(u) = b_s u + c_s u^2 + d_s u^3

  because each g_s vanishes at u=0 and the u=1 plateaus telescope to
  f(floor) - f(0) exactly; linear extrapolation outside the domain is the
  extra  -hS_0*relu(-t) + hS_{K-1}*relu(t-(K-1))  term.

  Per group of 128 rows (64 dims x 2 segments) the fields are built with
  four engine ops -- y = ACT Identity(t - s) (fp32->fp16), u = DVE
  clamp(y,0,1) (4x mode), u2 = DVE u*u, u3 = Pool u2*u -- and contracted
  against fp16 tables b/c/d in PSUM. t is replicated [t;t] host-side, so
  there is no per-group broadcast matmul.

Self-contained: hardcodes shapes B=8192, D=64, K=64, O=64, 8 cores.
"""

import sys

import numpy as np

sys.path.insert(0, "/opt/trn_rl_repo")

from concourse import bass, mybir  # noqa: E402
from concourse.bass_utils import run_bass_kernel_spmd  # noqa: E402
from concourse.tile import TileContext  # noqa: E402

F32 = mybir.dt.float32
F16 = mybir.dt.float16
F8 = mybir.dt.float8e4
ALU = mybir.AluOpType
AF = mybir.ActivationFunctionType
PM = mybir.MatmulPerfMode

B, D, K, O = 8192, 64, 64, 64
NCORES = 8
BSH = B // NCORES          # 1024 batch rows per core
NCHUNK = 2                 # 512-column matmul chunks
CHUNK = BSH // NCHUNK      # 512
NS = K - 1                 # 63 segments
NGRP = 32                  # groups of 2 segments (last half padded)
X_MIN, X_MAX = -2.0, 2.0
H = (X_MAX - X_MIN) / (K - 1)

CTB = NGRP * 3 * O         # 6144 table cols: per group [b | c | d] x O
TB_SPLIT = 8 * 3 * O       # first-chunk table DMA (groups 0..7)

# sb const tensor [128, 34] fp32: cols 0..31 group biases (-s per partition),
# col 32 = -(K-1) edge-hi bias, col 33 = 0.0 edge-lo bias
SB_EHI = 32
SB_ELO = 33
CSB = 34

WORK_BUFS = 4
WARM_N = 9                 # PE p-state warm matmuls bridging the DMA wait
EDGE_AT = 8                # group index after which edge fields are built
U3_DVE = {2, 4, 6, 10, 12, 14, 18, 20, 22, 26, 28, 30}  # u3 on DVE
U2_POOL = set()            # fp16 groups whose u2 runs on Pool to unload DVE
Y_DVE = set()              # groups whose y runs on DVE (ts, 2x_2p) not ACT
USE_POW = False            # pow not supported by walrus codegen
# Groups evaluated via fp8-e4m3 DoubleRow: fields q=u(u-1), r=q*u (zero on
# both plateaus, so fp8 tables only touch the locally-active segment) with
# tables (c+d, d); the u-field stays fp16 with table dC. Interleaved with
# fp16 groups so Pool's two fp8 writes per DR group pipeline against ACT's
# y cadence.
DR_GROUPS = frozenset(range(1, 32, 2))
NDR = len(DR_GROUPS)
# group emission order: group 0 must stay first (PSUM start + halved DMA
# wait); ending on an fp16 group whose u3 is on DVE keeps Pool off the
# final dependency chain
GROUP_ORDER = list(range(32))
# engines for the four 256-col output pieces (ACT / DVE / Pool)
OUT_ENGINES = ("act", "dve", "act", "dve")

TRACE = False
LAST_EXEC_NS = None


def _pchip_slopes_uniform(y, h):
    """numpy float32 port of reference._pchip_slopes_uniform. y: [..., K]."""
    y = y.astype(np.float32)
    delta = ((y[..., 1:] - y[..., :-1]) / np.float32(h)).astype(np.float32)
    dp, dn = delta[..., :-1], delta[..., 1:]
    same_sign = dp * dn > 0
    d_mid = np.where(
        same_sign, (2.0 * dp * dn / (dp + dn + np.float32(1e-12))), np.float32(0.0)
    ).astype(np.float32)

    def _fix_endpoint(d_end, delta0, delta1):
        d_end = np.where(d_end * delta0 <= 0, np.float32(0.0), d_end)
        d_end = np.where(
            (delta0 * delta1 < 0) & (np.abs(d_end) > 3.0 * np.abs(delta0)),
            (3.0 * delta0).astype(np.float32),
            d_end,
        )
        return d_end.astype(np.float32)

    d0 = _fix_endpoint(
        ((3.0 * delta[..., 0] - delta[..., 1]) / 2.0).astype(np.float32),
        delta[..., 0],
        delta[..., 1],
    )
    dN = _fix_endpoint(
        ((3.0 * delta[..., -1] - delta[..., -2]) / 2.0).astype(np.float32),
        delta[..., -1],
        delta[..., -2],
    )
    return np.concatenate([d0[..., None], d_mid, dN[..., None]], axis=-1)


def _build_kernel():
    nc = bass.Bass()

    t2 = nc.declare_dram_parameter("t2", [128, BSH], F32, isOutput=False)
    tb = nc.declare_dram_parameter("tb", [128, CTB], F16, isOutput=False)
    tb8 = nc.declare_dram_parameter("tb8", [128, 2, NDR * O], F8, isOutput=False)
    etab = nc.declare_dram_parameter("etab", [128, O], F16, isOutput=False)
    sb = nc.declare_dram_parameter("sb", [128, CSB], F32, isOutput=False)
    k0 = nc.declare_dram_parameter("k0", [O, 1], F32, isOutput=False)
    outt = nc.declare_dram_parameter("outt", [O, BSH], F32, isOutput=True)

    with TileContext(nc) as tc:
        with (
            tc.tile_pool(name="consts", bufs=1) as consts,
            tc.tile_pool(name="work", bufs=WORK_BUFS) as work,
            tc.tile_pool(name="accp", bufs=1, space="PSUM") as accp,
        ):
            t2_sb = consts.tile([128, BSH], F32)
            tb_sb = consts.tile([128, CTB], F16)
            tb8_sb = consts.tile([128, 2, NDR * O], F8)
            etab_sb = consts.tile([128, O], F16)
            sb_sb = consts.tile([128, CSB], F32)
            k0_sb = consts.tile([O, 1], F32)
            # sb + first table chunk serially on the SP queue; t2 halves on
            # the DVE/ACT queues in parallel so group 0 starts ~1us earlier
            nc.sync.dma_start(sb_sb[:], sb[:])
            nc.scalar.dma_start(t2_sb[:, 0:CHUNK], t2[:, 0:CHUNK])
            nc.gpsimd.dma_start(t2_sb[:, CHUNK:], t2[:, CHUNK:])
            nc.sync.dma_start(tb_sb[:, :TB_SPLIT], tb[:, :TB_SPLIT])
            nc.sync.dma_start(tb8_sb[:], tb8[:])
            nc.sync.dma_start(tb_sb[:, TB_SPLIT:], tb[:, TB_SPLIT:])
            nc.sync.dma_start(etab_sb[:], etab[:])
            nc.sync.dma_start(k0_sb[:], k0[:])

            dr_list = sorted(DR_GROUPS)

            def grp_tab(j, f):
                lo = j * 3 * O + f * O
                return tb_sb[:, lo : lo + O]

            def dr_tab(j):
                gi = dr_list.index(j)
                return tb8_sb[:, :, gi * O : (gi + 1) * O]

            # PSUM accumulator [O, 1024] (2 banks). Warm matmuls keep the PE
            # p-state ramp going from t=0 on a memset tile; results are
            # discarded by the start=True restarts below.
            # one PSUM tile per 512-col chunk so chunk 0's output path does
            # not serialize behind chunk 1's accumulation (tile-granularity
            # dependency tracking)
            acc0 = accp.tile([O, CHUNK], F32)
            acc1 = accp.tile([O, CHUNK], F32)
            accs = [acc0, acc1]
            warm = consts.tile([128, 512], F16, tag="warm")
            nc.vector.memset(warm[:], 0.0)
            # preload the activation-function table before t2 arrives so the
            # first y doesn't pay the 1283ns table load (separate output tile
            # so the warm matmuls below don't serialize behind it)
            dummy = consts.tile([1, 1], F16, tag="dummy")
            nc.scalar.activation(dummy[:], warm[0:1, 0:1], AF.Identity)
            for _ in range(WARM_N):
                nc.tensor.matmul(
                    acc0[0:64, 0:512],
                    warm[:, 0:64],
                    warm[:, 0:512],
                    start=True,
                    stop=True,
                )

            edges = consts.tile([128, BSH], F16, tag="edges")
            obs = []
            for q in range(4):
                ob_q = consts.tile([O, CHUNK // 2], F32, tag=f"ob{q}", name=f"ob{q}")
                obs.append(ob_q)

            for gidx, j in enumerate(GROUP_ORDER):
                is_dr = j in DR_GROUPS
                y = work.tile([128, BSH], F16, tag="y")
                u = work.tile([128, BSH], F16, tag="u")
                if is_dr:
                    qa = work.tile([128, BSH], F16, tag="qa")
                    qr = work.tile([128, 2, BSH], F8, tag="qr")
                else:
                    u2 = work.tile([128, BSH], F16, tag="u2")
                    u3 = work.tile([128, BSH], F16, tag="u3")
                # group 0 is built in column halves so its first matmuls only
                # wait on the first half of the t2 DMA
                halves = (
                    [slice(0, CHUNK), slice(CHUNK, BSH)]
                    if gidx == 0
                    else [slice(0, BSH)]
                )
                for h in halves:
                    if j in Y_DVE:
                        nc.vector.tensor_scalar(
                            y[:, h], t2_sb[:, h], sb_sb[:, j : j + 1], None,
                            ALU.add,
                        )
                    else:
                        nc.scalar.activation(
                            y[:, h], t2_sb[:, h], AF.Identity,
                            bias=sb_sb[:, j : j + 1], scale=1.0,
                        )
                    nc.vector.tensor_scalar(
                        u[:, h], y[:, h], 0.0, 1.0, ALU.max, ALU.min
                    )
                    if is_dr:
                        nc.vector.tensor_scalar(qa[:, h], u[:, h], -1.0, None, ALU.add)
                        nc.gpsimd.tensor_tensor(
                            qr[:, 0, h], u[:, h], qa[:, h], ALU.mult
                        )
                        nc.gpsimd.tensor_tensor(
                            qr[:, 1, h], qr[:, 0, h], u[:, h], ALU.mult
                        )
                    elif USE_POW:
                        nc.vector.tensor_scalar(u2[:, h], u[:, h], 2.0, None, ALU.pow)
                        if j in U3_DVE:
                            nc.vector.tensor_scalar(
                                u3[:, h], u[:, h], 3.0, None, ALU.pow
                            )
                        else:
                            nc.gpsimd.tensor_tensor(
                                u3[:, h], u2[:, h], u[:, h], ALU.mult
                            )
                    elif j in U2_POOL:
                        nc.gpsimd.tensor_tensor(u2[:, h], u[:, h], u[:, h], ALU.mult)
                        nc.gpsimd.tensor_tensor(u3[:, h], u2[:, h], u[:, h], ALU.mult)
                    else:
                        nc.vector.tensor_tensor(u2[:, h], u[:, h], u[:, h], ALU.mult)
                        if j in U3_DVE:
                            nc.vector.tensor_tensor(
                                u3[:, h], u2[:, h], u[:, h], ALU.mult
                            )
                        else:
                            nc.gpsimd.tensor_tensor(
                                u3[:, h], u2[:, h], u[:, h], ALU.mult
                            )

                if gidx == EDGE_AT:
                    # edge (extrapolation) fields, computed mid-stream where
                    # DVE has slack: rows 0:64 = relu(-t) -> -hS[d,0],
                    # rows 64:128 = relu(t-63) -> hS[d,63]
                    nc.vector.tensor_scalar(
                        edges[0:64, :], t2_sb[0:64, :], -1.0, 0.0,
                        ALU.mult, ALU.max,
                    )
                    nc.vector.tensor_scalar(
                        edges[64:128, :], t2_sb[64:128, :], float(-(K - 1)),
                        0.0, ALU.add, ALU.max,
                    )

                last = gidx == NGRP - 1
                for c in range(NCHUNK):
                    sl = slice(c * CHUNK, (c + 1) * CHUNK)
                    nc.tensor.matmul(
                        accs[c][:], grp_tab(j, 0), u[:, sl],
                        start=(gidx == 0), stop=False,
                    )
                    if is_dr:
                        nc.tensor.matmul(
                            accs[c][:], dr_tab(j), qr[:, :, sl],
                            start=False, stop=False, perf_mode=PM.DoubleRow,
                        )
                    else:
                        nc.tensor.matmul(
                            accs[c][:], grp_tab(j, 1), u2[:, sl],
                            start=False, stop=False,
                        )
                        nc.tensor.matmul(
                            accs[c][:], grp_tab(j, 2), u3[:, sl],
                            start=False, stop=False,
                        )
                    if last:
                        nc.tensor.matmul(
                            accs[c][:], etab_sb[:], edges[:, sl],
                            start=False, stop=True,
                        )

            # bias/const add + DMA out in 256-col pieces, after ALL matmuls
            # (acc is one tile: an early read would add a write-after-read
            # stall on the remaining accumulation). Separate ob tiles so the
            # four pieces don't serialize; DMAs spread across queues.
            dma_eng = [nc.sync, nc.gpsimd, nc.sync, nc.scalar]
            for q in range(4):
                qsl = slice(q * (CHUNK // 2), (q + 1) * (CHUNK // 2))
                asl = slice((q % 2) * (CHUNK // 2), (q % 2 + 1) * (CHUNK // 2))
                eng = OUT_ENGINES[q]
                if eng == "act":
                    nc.scalar.activation(
                        obs[q][:], accs[q // 2][:, asl], AF.Identity,
                        bias=k0_sb[:, 0:1], scale=1.0,
                    )
                elif eng == "dve":
                    nc.vector.tensor_scalar(
                        obs[q][:], accs[q // 2][:, asl], k0_sb[:, 0:1], None, ALU.add
                    )
                else:
                    nc.gpsimd.tensor_scalar(
                        obs[q][:], accs[q // 2][:, asl], k0_sb[:, 0:1], None, ALU.add
                    )
                dma_eng[q].dma_start(outt[:, qsl], obs[q][:])

    _split_multiwaits(nc)
    return nc


def _split_multiwaits(nc):
    """walrus (neuronx-cc) allows one sync wait per instruction; move extra
    waits onto standalone NoOps inserted just before the offender."""
    cnt = 0
    for f in nc.m.functions:
        for blk in f.blocks:
            out = []
            changed = False
            for ins in blk.instructions:
                si = ins.sync_info
                if si is not None and len(si.on_wait) > 1:
                    waits = list(si.on_wait)
                    for w in waits[:-1]:
                        nop = mybir.InstNoOp(name=f"I-ws-{cnt}", ins=[], outs=[])
                        cnt += 1
                        nop.engine = ins.engine
                        nop.sync_info = type(si)(on_wait=[w], on_update=[])
                        out.append(nop)
                    ins.sync_info = type(si)(
                        on_wait=[waits[-1]], on_update=list(si.on_update)
                    )
                    changed = True
                out.append(ins)
            if changed:
                blk.instructions = out


def _host_tables(coeffs, bias):
    coeffs = np.ascontiguousarray(np.asarray(coeffs, dtype=np.float32))
    bias = np.asarray(bias, dtype=np.float32)
    slopes = _pchip_slopes_uniform(coeffs, H)          # [O, D, K]
    hs = (slopes * np.float32(H)).astype(np.float32)   # h * S

    C = coeffs
    dC = C[..., 1:] - C[..., :-1]                      # [O, D, NS]
    c = (3.0 * dC - 2.0 * hs[..., :-1] - hs[..., 1:]).astype(np.float32)
    d = (-2.0 * dC + hs[..., :-1] + hs[..., 1:]).astype(np.float32)
    c16 = c.astype(np.float16)
    d16 = d.astype(np.float16)
    # compensate b so the u=1 plateau sum b+c+d telescopes to dC as exactly
    # as fp16 allows
    b16 = (dC - c16.astype(np.float32) - d16.astype(np.float32)).astype(np.float16)

    from ml_dtypes import float8_e4m3fn as E4M3

    tb = np.zeros((128, CTB), dtype=np.float16)
    tb8v = np.zeros((128, 2, NDR * O), dtype=np.float32)
    dr_list = sorted(DR_GROUPS)
    tabs = (b16, c16, d16)
    for j in range(NGRP):
        is_dr = j in DR_GROUPS
        for half in range(2):
            s = 2 * j + half
            if s >= NS:
                continue
            rows = slice(half * 64, (half + 1) * 64)
            if is_dr:
                # u-field table = dC (plateau-exact); q,r tables in fp8
                gi = dr_list.index(j)
                lo = j * 3 * O
                tb[rows, lo : lo + O] = dC[:, :, s].T.astype(np.float16)
                tb8v[rows, 0, gi * O : (gi + 1) * O] = (c + d)[:, :, s].T
                tb8v[rows, 1, gi * O : (gi + 1) * O] = d[:, :, s].T
            else:
                for f in range(3):
                    lo = j * 3 * O + f * O
                    # rows = dims, cols = o
                    tb[rows, lo : lo + O] = tabs[f][:, :, s].T
    tb8 = tb8v.astype(E4M3)

    etab = np.zeros((128, O), dtype=np.float16)
    etab[0:64, :] = -hs[:, :, 0].T
    etab[64:128, :] = hs[:, :, K - 1].T

    sb = np.zeros((128, CSB), dtype=np.float32)
    for j in range(NGRP):
        sb[0:64, j] = -(2 * j)
        sb[64:128, j] = -(2 * j + 1)
    sb[:, SB_EHI] = -(K - 1)
    sb[:, SB_ELO] = 0.0

    k0 = (C[..., 0].sum(axis=1) + bias).astype(np.float32).reshape(O, 1)
    return tb, tb8, etab, sb, k0


def kernel(x, coeffs, bias):
    global LAST_EXEC_NS
    x = np.asarray(x, dtype=np.float32)
    tb, tb8, etab, sb, k0 = _host_tables(coeffs, bias)

    in_maps = []
    for r in range(NCORES):
        xc = x[r * BSH : (r + 1) * BSH, :]             # [1024, 64]
        t = ((xc.T - np.float32(X_MIN)) * np.float32(1.0 / H)).astype(np.float32)
        t2 = np.ascontiguousarray(np.concatenate([t, t], axis=0))  # [128, 1024]
        in_maps.append(
            {"t2": t2, "tb": tb, "tb8": tb8, "etab": etab, "sb": sb, "k0": k0}
        )

    nc = _build_kernel()
    res = run_bass_kernel_spmd(nc, in_maps, list(range(NCORES)), trace=TRACE)
    LAST_EXEC_NS = getattr(res, "exec_time_ns", None)

    out = np.empty((B, O), dtype=np.float32)
    for r in range(NCORES):
        out_t = res.results[r]["outt"]                 # [O, 1024]
        out[r * BSH : (r + 1) * BSH, :] = np.asarray(out_t).T
    return out


if __name__ == "__main__":
    rng = np.random.default_rng(0)
    x = rng.standard_normal((B, D)).astype(np.float32)
    coeffs = (0.01 * rng.standard_normal((O, D, K))).astype(np.float32)
    bias = np.zeros((O,), dtype=np.float32)
    out = kernel(x, coeffs, bias)
    print("out", out.shape, out.dtype, float(np.abs(out).mean()))


# revision 42
# speedup vs baseline: 1.9724x; 1.0015x over previous
"""Trainium2 Bass kernel for a bare KAN layer (PCHIP spline mixing).

Math: out[b, o] = sum_d f_{o,d}(x[b,d]) + bias[o], where f_{o,d} is the PCHIP
cubic interpolant of coeffs[o,d,:] on K=64 uniform knots over [-2, 2], with
linear extrapolation outside.

Device strategy (per core, data-parallel over batch):
  Segment-power telescoping basis. With t = (x - X_MIN)/h and
  u_s = clamp(t - s, 0, 1) for segments s = 0..K-2:

      f(t) = f(0) + sum_s g_s(u_s),   g_s(u) = b_s u + c_s u^2 + d_s u^3

  because each g_s vanishes at u=0 and the u=1 plateaus telescope to
  f(floor) - f(0) exactly; linear extrapolation outside the domain is the
  extra  -hS_0*relu(-t) + hS_{K-1}*relu(t-(K-1))  term.

  Per group of 128 rows (64 dims x 2 segments) the fields are built with
  four engine ops -- y = ACT Identity(t - s) (fp32->fp16), u = DVE
  clamp(y,0,1) (4x mode), then either u2/u3 fp16 multiplies (DVE/Pool) or,
  for half the groups, localized fields q = u(u-1), r = q*u written as
  fp8-e4m3 and contracted with a DoubleRow matmul (0.5 cycles/row) against
  fp8 tables (c+d, d) -- q,r vanish on both plateaus, so fp8 error only
  touches the active segment. All fields accumulate into fp32 PSUM. t is
  replicated [t;t] host-side, so there is no per-group broadcast matmul.

Self-contained: hardcodes shapes B=8192, D=64, K=64, O=64, 8 cores.
"""

import sys

import numpy as np

sys.path.insert(0, "/opt/trn_rl_repo")

from concourse import bass, mybir  # noqa: E402
from concourse.bass_utils import run_bass_kernel_spmd  # noqa: E402
from concourse.tile import TileContext  # noqa: E402

F32 = mybir.dt.float32
F16 = mybir.dt.float16
F8 = mybir.dt.float8e4
ALU = mybir.AluOpType
AF = mybir.ActivationFunctionType
PM = mybir.MatmulPerfMode

B, D, K, O = 8192, 64, 64, 64
NCORES = 8
BSH = B // NCORES          # 1024 batch rows per core
NCHUNK = 2                 # 512-column matmul chunks
CHUNK = BSH // NCHUNK      # 512
NS = K - 1                 # 63 segments
NGRP = 32                  # groups of 2 segments (last half padded)
X_MIN, X_MAX = -2.0, 2.0
H = (X_MAX - X_MIN) / (K - 1)

CTB = NGRP * 3 * O         # 6144 table cols: per group [b | c | d] x O
TB_SPLIT = 8 * 3 * O       # first-chunk table DMA (groups 0..7)

# sb const tensor [128, 34] fp32: cols 0..31 group biases (-s per partition),
# col 32 = -(K-1) edge-hi bias, col 33 = 0.0 edge-lo bias
SB_EHI = 32
SB_ELO = 33
CSB = 34

WORK_BUFS = 4
WARM_N = 9                 # PE p-state warm matmuls bridging the DMA wait
EDGE_AT = 8                # group index after which edge fields are built
U3_DVE = {2, 4, 6, 10, 12, 14, 18, 20, 22, 26, 28, 30}  # u3 on DVE
U2_POOL = set()            # fp16 groups whose u2 runs on Pool to unload DVE
Y_DVE = set()              # groups whose y runs on DVE (ts, 2x_2p) not ACT
USE_POW = False            # pow not supported by walrus codegen
# Groups evaluated via fp8-e4m3 DoubleRow: fields q=u(u-1), r=q*u (zero on
# both plateaus, so fp8 tables only touch the locally-active segment) with
# tables (c+d, d); the u-field stays fp16 with table dC. Interleaved with
# fp16 groups so Pool's two fp8 writes per DR group pipeline against ACT's
# y cadence.
DR_GROUPS = frozenset(range(1, 32, 2))
NDR = len(DR_GROUPS)
# group emission order: group 0 must stay first (PSUM start + halved DMA
# wait); ending on an fp16 group whose u3 is on DVE keeps Pool off the
# final dependency chain
GROUP_ORDER = list(range(32))
# engines for the four 256-col output pieces (ACT / DVE / Pool)
OUT_ENGINES = ("act", "dve", "act", "dve")

TRACE = False
LAST_EXEC_NS = None


def _pchip_slopes_uniform(y, h):
    """numpy float32 port of reference._pchip_slopes_uniform. y: [..., K]."""
    y = y.astype(np.float32)
    delta = ((y[..., 1:] - y[..., :-1]) / np.float32(h)).astype(np.float32)
    dp, dn = delta[..., :-1], delta[..., 1:]
    same_sign = dp * dn > 0
    d_mid = np.where(
        same_sign, (2.0 * dp * dn / (dp + dn + np.float32(1e-12))), np.float32(0.0)
    ).astype(np.float32)

    def _fix_endpoint(d_end, delta0, delta1):
        d_end = np.where(d_end * delta0 <= 0, np.float32(0.0), d_end)
        d_end = np.where(
            (delta0 * delta1 < 0) & (np.abs(d_end) > 3.0 * np.abs(delta0)),
            (3.0 * delta0).astype(np.float32),
            d_end,
        )
        return d_end.astype(np.float32)

    d0 = _fix_endpoint(
        ((3.0 * delta[..., 0] - delta[..., 1]) / 2.0).astype(np.float32),
        delta[..., 0],
        delta[..., 1],
    )
    dN = _fix_endpoint(
        ((3.0 * delta[..., -1] - delta[..., -2]) / 2.0).astype(np.float32),
        delta[..., -1],
        delta[..., -2],
    )
    return np.concatenate([d0[..., None], d_mid, dN[..., None]], axis=-1)


def _build_kernel():
    nc = bass.Bass()

    t2 = nc.declare_dram_parameter("t2", [128, BSH], F32, isOutput=False)
    tb = nc.declare_dram_parameter("tb", [128, CTB], F16, isOutput=False)
    tb8 = nc.declare_dram_parameter("tb8", [128, 2, NDR * O], F8, isOutput=False)
    etab = nc.declare_dram_parameter("etab", [128, O], F16, isOutput=False)
    sb = nc.declare_dram_parameter("sb", [128, CSB], F32, isOutput=False)
    k0 = nc.declare_dram_parameter("k0", [O, 1], F32, isOutput=False)
    outt = nc.declare_dram_parameter("outt", [O, BSH], F32, isOutput=True)

    with TileContext(nc) as tc:
        with (
            tc.tile_pool(name="consts", bufs=1) as consts,
            tc.tile_pool(name="work", bufs=WORK_BUFS) as work,
            tc.tile_pool(name="accp", bufs=1, space="PSUM") as accp,
        ):
            t2_sb = consts.tile([128, BSH], F32)
            tb_sb = consts.tile([128, CTB], F16)
            tb8_sb = consts.tile([128, 2, NDR * O], F8)
            etab_sb = consts.tile([128, O], F16)
            sb_sb = consts.tile([128, CSB], F32)
            k0_sb = consts.tile([O, 1], F32)
            # sb + first table chunk serially on the SP queue; t2 halves on
            # the DVE/ACT queues in parallel so group 0 starts ~1us earlier
            nc.sync.dma_start(sb_sb[:], sb[:])
            nc.scalar.dma_start(t2_sb[:, 0:CHUNK], t2[:, 0:CHUNK])
            nc.gpsimd.dma_start(t2_sb[:, CHUNK:], t2[:, CHUNK:])
            nc.sync.dma_start(tb_sb[:, :TB_SPLIT], tb[:, :TB_SPLIT])
            nc.sync.dma_start(tb8_sb[:], tb8[:])
            nc.sync.dma_start(tb_sb[:, TB_SPLIT:], tb[:, TB_SPLIT:])
            nc.sync.dma_start(etab_sb[:], etab[:])
            nc.sync.dma_start(k0_sb[:], k0[:])

            dr_list = sorted(DR_GROUPS)

            def grp_tab(j, f):
                lo = j * 3 * O + f * O
                return tb_sb[:, lo : lo + O]

            def dr_tab(j):
                gi = dr_list.index(j)
                return tb8_sb[:, :, gi * O : (gi + 1) * O]

            # PSUM accumulator [O, 1024] (2 banks). Warm matmuls keep the PE
            # p-state ramp going from t=0 on a memset tile; results are
            # discarded by the start=True restarts below.
            # one PSUM tile per 512-col chunk so chunk 0's output path does
            # not serialize behind chunk 1's accumulation (tile-granularity
            # dependency tracking)
            acc0 = accp.tile([O, CHUNK], F32)
            acc1 = accp.tile([O, CHUNK], F32)
            accs = [acc0, acc1]
            warm = consts.tile([128, 512], F16, tag="warm")
            nc.vector.memset(warm[:], 0.0)
            # preload the activation-function table before t2 arrives so the
            # first y doesn't pay the 1283ns table load (separate output tile
            # so the warm matmuls below don't serialize behind it)
            dummy = consts.tile([1, 1], F16, tag="dummy")
            nc.scalar.activation(dummy[:], warm[0:1, 0:1], AF.Identity)
            for _ in range(WARM_N):
                nc.tensor.matmul(
                    acc0[0:64, 0:512],
                    warm[:, 0:64],
                    warm[:, 0:512],
                    start=True,
                    stop=True,
                )

            edges = consts.tile([128, BSH], F16, tag="edges")
            obs = []
            ob_sizes = [256, 256, 320, 192]
            for q in range(4):
                ob_q = consts.tile(
                    [O, ob_sizes[q]], F32, tag=f"ob{q}", name=f"ob{q}"
                )
                obs.append(ob_q)

            for gidx, j in enumerate(GROUP_ORDER):
                is_dr = j in DR_GROUPS
                y = work.tile([128, BSH], F16, tag="y")
                u = work.tile([128, BSH], F16, tag="u")
                if is_dr:
                    qa = work.tile([128, BSH], F16, tag="qa")
                    qr = work.tile([128, 2, BSH], F8, tag="qr")
                else:
                    u2 = work.tile([128, BSH], F16, tag="u2")
                    u3 = work.tile([128, BSH], F16, tag="u3")
                # group 0 is built in column halves so its first matmuls only
                # wait on the first half of the t2 DMA
                halves = (
                    [slice(0, CHUNK), slice(CHUNK, BSH)]
                    if gidx == 0
                    else [slice(0, BSH)]
                )
                for h in halves:
                    if j in Y_DVE:
                        nc.vector.tensor_scalar(
                            y[:, h], t2_sb[:, h], sb_sb[:, j : j + 1], None,
                            ALU.add,
                        )
                    else:
                        nc.scalar.activation(
                            y[:, h], t2_sb[:, h], AF.Identity,
                            bias=sb_sb[:, j : j + 1], scale=1.0,
                        )
                    nc.vector.tensor_scalar(
                        u[:, h], y[:, h], 0.0, 1.0, ALU.max, ALU.min
                    )
                    if is_dr:
                        nc.vector.tensor_scalar(qa[:, h], u[:, h], -1.0, None, ALU.add)
                        nc.gpsimd.tensor_tensor(
                            qr[:, 0, h], u[:, h], qa[:, h], ALU.mult
                        )
                        nc.gpsimd.tensor_tensor(
                            qr[:, 1, h], qr[:, 0, h], u[:, h], ALU.mult
                        )
                    elif USE_POW:
                        nc.vector.tensor_scalar(u2[:, h], u[:, h], 2.0, None, ALU.pow)
                        if j in U3_DVE:
                            nc.vector.tensor_scalar(
                                u3[:, h], u[:, h], 3.0, None, ALU.pow
                            )
                        else:
                            nc.gpsimd.tensor_tensor(
                                u3[:, h], u2[:, h], u[:, h], ALU.mult
                            )
                    elif j in U2_POOL:
                        nc.gpsimd.tensor_tensor(u2[:, h], u[:, h], u[:, h], ALU.mult)
                        nc.gpsimd.tensor_tensor(u3[:, h], u2[:, h], u[:, h], ALU.mult)
                    else:
                        nc.vector.tensor_tensor(u2[:, h], u[:, h], u[:, h], ALU.mult)
                        if j in U3_DVE:
                            nc.vector.tensor_tensor(
                                u3[:, h], u2[:, h], u[:, h], ALU.mult
                            )
                        else:
                            nc.gpsimd.tensor_tensor(
                                u3[:, h], u2[:, h], u[:, h], ALU.mult
                            )

                if gidx == EDGE_AT:
                    # edge (extrapolation) fields, computed mid-stream where
                    # DVE has slack: rows 0:64 = relu(-t) -> -hS[d,0],
                    # rows 64:128 = relu(t-63) -> hS[d,63]
                    nc.vector.tensor_scalar(
                        edges[0:64, :], t2_sb[0:64, :], -1.0, 0.0,
                        ALU.mult, ALU.max,
                    )
                    nc.vector.tensor_scalar(
                        edges[64:128, :], t2_sb[64:128, :], float(-(K - 1)),
                        0.0, ALU.add, ALU.max,
                    )

                last = gidx == NGRP - 1
                for c in range(NCHUNK):
                    sl = slice(c * CHUNK, (c + 1) * CHUNK)
                    nc.tensor.matmul(
                        accs[c][:], grp_tab(j, 0), u[:, sl],
                        start=(gidx == 0), stop=False,
                    )
                    if is_dr:
                        nc.tensor.matmul(
                            accs[c][:], dr_tab(j), qr[:, :, sl],
                            start=False, stop=False, perf_mode=PM.DoubleRow,
                        )
                    else:
                        nc.tensor.matmul(
                            accs[c][:], grp_tab(j, 1), u2[:, sl],
                            start=False, stop=False,
                        )
                        nc.tensor.matmul(
                            accs[c][:], grp_tab(j, 2), u3[:, sl],
                            start=False, stop=False,
                        )
                    if last:
                        nc.tensor.matmul(
                            accs[c][:], etab_sb[:], edges[:, sl],
                            start=False, stop=True,
                        )

            # bias/const add + DMA out in 256-col pieces, after ALL matmuls
            # (acc is one tile: an early read would add a write-after-read
            # stall on the remaining accumulation). Separate ob tiles so the
            # four pieces don't serialize; DMAs spread across queues.
            dma_eng = [nc.sync, nc.gpsimd, nc.sync, nc.scalar]
            # piece boundaries: last piece smallest so the final DMA (on the
            # critical path) has the shortest transfer
            bounds = [0, 256, 512, 832, 1024]
            for q in range(4):
                qsl = slice(bounds[q], bounds[q + 1])
                asl = slice(bounds[q] % CHUNK, ((bounds[q + 1] - 1) % CHUNK) + 1)
                eng = OUT_ENGINES[q]
                if eng == "act":
                    nc.scalar.activation(
                        obs[q][:], accs[q // 2][:, asl], AF.Identity,
                        bias=k0_sb[:, 0:1], scale=1.0,
                    )
                elif eng == "dve":
                    nc.vector.tensor_scalar(
                        obs[q][:], accs[q // 2][:, asl], k0_sb[:, 0:1], None, ALU.add
                    )
                else:
                    nc.gpsimd.tensor_scalar(
                        obs[q][:], accs[q // 2][:, asl], k0_sb[:, 0:1], None, ALU.add
                    )
                dma_eng[q].dma_start(outt[:, qsl], obs[q][:])

    _split_multiwaits(nc)
    return nc


def _split_multiwaits(nc):
    """walrus (neuronx-cc) allows one sync wait per instruction; move extra
    waits onto standalone NoOps inserted just before the offender."""
    cnt = 0
    for f in nc.m.functions:
        for blk in f.blocks:
            out = []
            changed = False
            for ins in blk.instructions:
                si = ins.sync_info
                if si is not None and len(si.on_wait) > 1:
                    waits = list(si.on_wait)
                    for w in waits[:-1]:
                        nop = mybir.InstNoOp(name=f"I-ws-{cnt}", ins=[], outs=[])
                        cnt += 1
                        nop.engine = ins.engine
                        nop.sync_info = type(si)(on_wait=[w], on_update=[])
                        out.append(nop)
                    ins.sync_info = type(si)(
                        on_wait=[waits[-1]], on_update=list(si.on_update)
                    )
                    changed = True
                out.append(ins)
            if changed:
                blk.instructions = out


def _host_tables(coeffs, bias):
    coeffs = np.ascontiguousarray(np.asarray(coeffs, dtype=np.float32))
    bias = np.asarray(bias, dtype=np.float32)
    slopes = _pchip_slopes_uniform(coeffs, H)          # [O, D, K]
    hs = (slopes * np.float32(H)).astype(np.float32)   # h * S

    C = coeffs
    dC = C[..., 1:] - C[..., :-1]                      # [O, D, NS]
    c = (3.0 * dC - 2.0 * hs[..., :-1] - hs[..., 1:]).astype(np.float32)
    d = (-2.0 * dC + hs[..., :-1] + hs[..., 1:]).astype(np.float32)
    c16 = c.astype(np.float16)
    d16 = d.astype(np.float16)
    # compensate b so the u=1 plateau sum b+c+d telescopes to dC as exactly
    # as fp16 allows
    b16 = (dC - c16.astype(np.float32) - d16.astype(np.float32)).astype(np.float16)

    from ml_dtypes import float8_e4m3fn as E4M3

    tb = np.zeros((128, CTB), dtype=np.float16)
    tb8v = np.zeros((128, 2, NDR * O), dtype=np.float32)
    dr_list = sorted(DR_GROUPS)
    tabs = (b16, c16, d16)
    for j in range(NGRP):
        is_dr = j in DR_GROUPS
        for half in range(2):
            s = 2 * j + half
            if s >= NS:
                continue
            rows = slice(half * 64, (half + 1) * 64)
            if is_dr:
                # u-field table = dC (plateau-exact); q,r tables in fp8
                gi = dr_list.index(j)
                lo = j * 3 * O
                tb[rows, lo : lo + O] = dC[:, :, s].T.astype(np.float16)
                tb8v[rows, 0, gi * O : (gi + 1) * O] = (c + d)[:, :, s].T
                tb8v[rows, 1, gi * O : (gi + 1) * O] = d[:, :, s].T
            else:
                for f in range(3):
                    lo = j * 3 * O + f * O
                    # rows = dims, cols = o
                    tb[rows, lo : lo + O] = tabs[f][:, :, s].T
    tb8 = tb8v.astype(E4M3)

    etab = np.zeros((128, O), dtype=np.float16)
    etab[0:64, :] = -hs[:, :, 0].T
    etab[64:128, :] = hs[:, :, K - 1].T

    sb = np.zeros((128, CSB), dtype=np.float32)
    for j in range(NGRP):
        sb[0:64, j] = -(2 * j)
        sb[64:128, j] = -(2 * j + 1)
    sb[:, SB_EHI] = -(K - 1)
    sb[:, SB_ELO] = 0.0

    k0 = (C[..., 0].sum(axis=1) + bias).astype(np.float32).reshape(O, 1)
    return tb, tb8, etab, sb, k0


def kernel(x, coeffs, bias):
    global LAST_EXEC_NS
    x = np.asarray(x, dtype=np.float32)
    tb, tb8, etab, sb, k0 = _host_tables(coeffs, bias)

    in_maps = []
    for r in range(NCORES):
        xc = x[r * BSH : (r + 1) * BSH, :]             # [1024, 64]
        t = ((xc.T - np.float32(X_MIN)) * np.float32(1.0 / H)).astype(np.float32)
        t2 = np.ascontiguousarray(np.concatenate([t, t], axis=0))  # [128, 1024]
        in_maps.append(
            {"t2": t2, "tb": tb, "tb8": tb8, "etab": etab, "sb": sb, "k0": k0}
        )

    nc = _build_kernel()
    res = run_bass_kernel_spmd(nc, in_maps, list(range(NCORES)), trace=TRACE)
    LAST_EXEC_NS = getattr(res, "exec_time_ns", None)

    out = np.empty((B, O), dtype=np.float32)
    for r in range(NCORES):
        out_t = res.results[r]["outt"]                 # [O, 1024]
        out[r * BSH : (r + 1) * BSH, :] = np.asarray(out_t).T
    return out


if __name__ == "__main__":
    rng = np.random.default_rng(0)
    x = rng.standard_normal((B, D)).astype(np.float32)
    coeffs = (0.01 * rng.standard_normal((O, D, K))).astype(np.float32)
    bias = np.zeros((O,), dtype=np.float32)
    out = kernel(x, coeffs, bias)
    print("out", out.shape, out.dtype, float(np.abs(out).mean()))
